# revision 12
# baseline (speedup 1.0000x reference)
"""Trainium2 Bass kernel for nn_DuhamelLayer (8-channel long-FIR conv1d).

Math: out[b,o,t] = sum_k irf[o,k] * x[b, t+k-pad]  (cross-correlation,
'SAME' padding, pad = MAXK//2).  The conv is recast as a chain of
PSUM-accumulating 128x128 Toeplitz-block matmuls on the TensorEngine:

  t = 128*a + p,  k' = 128*c + (u - p)          (k' = k + GSHIFT)
  out[p, a] = sum_c sum_u M_c[u, p] * X[u, a + c]
  M_c[u, p] = w'[128*c + u - p]                 (dense Toeplitz block)
  X[u, m]   = xpad[128*m + u]                   (partition-fast layout)

GSHIFT=76 aligns the per-channel nonzero tap spans to 128-boundaries,
giving the provably minimal 62 blocks (= sum_o ceil((W_o+127)/128);
each tap k' is touched by both 128-blocks (k'+p)//128 as p sweeps, so
the block cover is forced once the phase is chosen).
Operands are bf16 (PE streams 1 col/cycle @2.4GHz warm, FWL weight
loads overlap the previous matmul); PSUM accumulates fp32; the output
is stored bf16 and widened on the host (rel_l2 vs fp64 ~3e-3).
Sharding: data-parallel over batch, 2 batches per core x 8 cores.

Schedule notes (from NTFF traces):
- No nc.Block: all five engine streams are straight-line code in the
  root basic block.  This removes the Block-exit all-engine barrier,
  letting each engine fall into the walrus epilogue (which clears a
  ~50-semaphore stripe per engine, ~2-5.5us!) as soon as its own
  stream ends, overlapping the other engines' tails.  All kernel
  semaphores that receive DMA receipts land in the DVE/Sync stripes,
  whose sweeps start only after those engines' streams end, so no
  receipt can race its own stripe's clear into a deadlock.
- Input x rides the Sync HWDGE ring; ALL weights ride the Scalar
  HWDGE ring, issued immediately (the two rings share the 16 SDMA
  engines at packet granularity, so x_b0 and the first weights stream
  concurrently).  Real matmuls start ~1.7us earlier than with the
  bundled single-ring scheme, and the mid-stream weight-arrival
  stalls at the ch5/ch4 boundaries disappear.
- NWARM warm-up matmuls on uninitialized SBUF bridge the input-DMA
  wait and un-gate the PE-HAM clock (cold 1.2GHz -> warm 2.4GHz after
  ~3.4us of busy PE).
"""

import numpy as np

# ---- static config (mirrors the nn.Module) ----
OMEGAS = [5.0, 7.0, 9.0, 12.0, 16.0, 22.0, 30.0, 40.0]
XI = 0.05
DT = 0.01
UJ_U1 = 0.01

_decay = (1.0 / (2.0 * np.pi * XI)) * np.log(1.0 / UJ_U1)
VALID_W = [int(2.0 * np.pi / w / np.sqrt(1.0 - XI**2) * _decay / DT) for w in OMEGAS]
KER = [2 * a - 1 for a in VALID_W]
MAXK = max(KER)          # 3687
OUT_CH = len(OMEGAS)     # 8
PAD = MAXK // 2          # 1843

B = 16                   # batch
T = 65536                # sequence length
NCORES = 8
BPC = B // NCORES        # 2 batches per core
A = T // 128             # 512 output columns per (b, o) tile

GSHIFT = 76              # global tap shift: minimizes total Toeplitz blocks
MM_DTYPE = "bfloat16"    # "bfloat16" | "float32r" | "float32"
OUT_DTYPE = "bfloat16"   # device-side output dtype ("bfloat16" | "float32")
MODE = "raw"             # kept for test.py compat
NWARM = 3                # coarse warm-up matmuls (N=512); see NWARM_FINE
NWARM_FINE = 9           # fine warm-up matmuls (N=128) after the coarse ones:
                         # ~107ns granularity so the PE never idles between
                         # warm-up end and input arrival — ANY sub-us PE gap
                         # before the HAM flip postpones the 2.4GHz clock by
                         # up to a whole 3.4us window (costs ~1.5-3us)
FINAL_WAITS = False      # wait out-DMA receipts before stream end (walrus
                         # postamble dma-drain covers them)
TRACE = False            # test.py flips this for profiling
TRACE_KWARGS = {}
LAST_RESULTS = None

_NC_CACHE = {}


def _build_wbank(log_omegas):
    """float32 numpy mirror of the reference's _build_irfs -> [OUT_CH, MAXK]."""
    lo = np.asarray(log_omegas, dtype=np.float32)
    omegas = np.clip(np.exp(lo), 0.01, 1000.0).astype(np.float32)
    sq = np.float32(np.sqrt(np.float32(1.0 - XI**2)))
    rows = []
    for i in range(OUT_CH):
        W, K = VALID_W[i], KER[i]
        tt = (np.arange(W, dtype=np.float32) * np.float32(DT)).astype(np.float32)
        omegaD = np.float32(omegas[i] * sq)
        irf = (
            (np.float32(1.0) / omegaD)
            * np.exp((-np.float32(XI) * omegas[i]) * tt)
            * np.sin(omegaD * tt)
        ).astype(np.float32)
        w = np.concatenate([irf[::-1], np.zeros((K // 2,), np.float32)])
        addpad = MAXK - K
        w = np.pad(w, (addpad // 2, addpad // 2))
        rows.append(w)
    return np.stack(rows)


def _plan_blocks(wbank_s):
    """Per channel, the Toeplitz block indices c spanning the nonzero taps."""
    blocks = []
    for o in range(OUT_CH):
        nz = np.nonzero(wbank_s[o])[0]
        kmin, kmax = int(nz.min()), int(nz.max())
        blocks.append(list(range(kmin // 128, (kmax + 127) // 128 + 1)))
    return blocks


def _build_weight_mats(wbank_s, blocks, np_dtype):
    """Per channel: [128, nblk*128] with column block i = M_{c_i}[u, p]."""
    maxk = wbank_s.shape[1]
    u = np.arange(128)[:, None]
    p = np.arange(128)[None, :]
    mats = []
    for o in range(OUT_CH):
        cols = []
        for c in blocks[o]:
            idx = 128 * c + u - p
            valid = (idx >= 0) & (idx < maxk)
            cols.append(
                np.where(valid, wbank_s[o][np.clip(idx, 0, maxk - 1)], np.float32(0.0))
            )
        mats.append(
            np.ascontiguousarray(np.concatenate(cols, axis=1)).astype(np_dtype)
        )
    return mats


def _build_nc_raw(blocks, xcols, mm_dtype, out_dtype):
    """Straight-line raw bacc kernel: DMA in, Toeplitz matmul chain, DMA out.

    No nc.Block: per-engine streams are emitted directly into the root
    basic block so no exit barrier separates the body from the walrus
    epilogue (see module docstring).
    """
    import concourse.bacc as bacc
    import concourse.mybir as mybir

    mm_dt = getattr(mybir.dt, mm_dtype)
    out_dt = getattr(mybir.dt, out_dtype)
    f32 = mybir.dt.float32

    nc = bacc.Bacc("TRN2", target_bir_lowering=False, debug=False)
    order = sorted(range(OUT_CH), key=lambda o: len(blocks[o]))

    # x batch 0 is the critical-path input: split it across BOTH HWDGE
    # rings (sync + scalar) so the two halves transfer concurrently.
    XSPLIT = 272
    xt0a_d = nc.dram_tensor("xt0a", [128, XSPLIT], mm_dt, kind="ExternalInput")
    xt0b_d = nc.dram_tensor("xt0b", [128, xcols - XSPLIT], mm_dt, kind="ExternalInput")
    xt1_d = nc.dram_tensor("xt1", [128, xcols], mm_dt, kind="ExternalInput")
    w_d = {
        o: nc.dram_tensor(f"wt{o}", [128, len(blocks[o]) * 128], mm_dt, kind="ExternalInput")
        for o in range(OUT_CH)
    }
    y_d = nc.dram_tensor("y", [BPC, OUT_CH, 128, A], out_dt, kind="ExternalOutput")

    NSLOT = 4  # psum slots; slot s holds banks (b0, b1) of channel k=s mod 4

    from contextlib import ExitStack

    with ExitStack() as ctx:
        xt0 = ctx.enter_context(nc.sbuf_tensor("xt0s", [128, xcols], mm_dt))
        xt1 = ctx.enter_context(nc.sbuf_tensor("xt1s", [128, xcols], mm_dt))
        warm = ctx.enter_context(nc.sbuf_tensor("warms", [128, 128 + A], mm_dt))
        wts = {
            o: ctx.enter_context(
                nc.sbuf_tensor(f"wts{o}", [128, len(blocks[o]) * 128], mm_dt)
            )
            for o in range(OUT_CH)
        }
        ots = [
            ctx.enter_context(nc.sbuf_tensor(f"ots{j}", [128, A], out_dt))
            for j in range(4)
        ]
        pss = [
            [
                ctx.enter_context(nc.psum_tensor(f"rps{s}_{b}", [128, A], f32))
                for b in range(BPC)
            ]
            for s in range(NSLOT)
        ]

        def wslice(o, i):
            return wts[o][:, i * 128 : (i + 1) * 128]

        def xslice(b, c):
            xt = xt0 if b == 0 else xt1
            return xt[:, c : c + A]

        # one semaphore per DMA: the 16 SDMA engines complete their shares of
        # successive same-ring DMAs out of order, so cumulative thresholds on
        # a shared semaphore do NOT imply per-DMA completion.
        xsa = ctx.enter_context(nc.semaphore("xsa"))
        xsb = ctx.enter_context(nc.semaphore("xsb"))
        xs1 = ctx.enter_context(nc.semaphore("xs1"))
        wsem = {o: ctx.enter_context(nc.semaphore(f"ws{o}")) for o in range(OUT_CH)}
        osem = [
            ctx.enter_context(nc.semaphore(f"os{i}")) for i in range(2 * OUT_CH + 1)
        ]
        mm_done = ctx.enter_context(nc.semaphore("mm_done"))
        copy_done_v = ctx.enter_context(nc.semaphore("copy_done_v"))

        KL = OUT_CH - 1
        H1 = 384   # first piece of the final half-chain (overlaps last copy)
        H2 = A - H1  # small final piece -> short last copy + out-DMA tail

        # chain schedule: the first two channels' b0 chains run before any
        # b1 chain, buying the xs1 (x_b1) receipt extra slack.  mm_done /
        # copy_done_v counts follow this order on every engine.
        sched = [(0, 0), (1, 0), (0, 1), (1, 1)]
        for _k in range(2, OUT_CH):
            sched += [(_k, 0), (_k, 1)]
        pos = {kb: i + 1 for i, kb in enumerate(sched)}

        # --- Sync engine: x input DMAs, then output DMAs gated on copies.
        nc.sync.dma_start(xt0[:, :XSPLIT], xt0a_d[:]).then_inc(xsa, 16)
        nc.sync.dma_start(xt1[:], xt1_d[:]).then_inc(xs1, 16)
        for k, b in sched:
            if (k, b) == (KL, 1):
                # last channel's b=1 pieces (copies are incs 16 and 17)
                for h, (off, w_, os_i) in enumerate(
                    ((0, H1, 2 * KL + 1), (H1, H2, 2 * OUT_CH))
                ):
                    nc.sync.wait_ge(copy_done_v, pos[(KL, 1)] + h)
                    nc.sync.dma_start(
                        y_d[1, order[KL]][:, off : off + w_],
                        ots[(2 * KL + 1) % 4][:, off : off + w_],
                    ).then_inc(osem[os_i], 16)
                continue
            nc.sync.wait_ge(copy_done_v, pos[(k, b)])
            nc.sync.dma_start(
                y_d[b, order[k]], ots[(2 * k + b) % 4][:]
            ).then_inc(osem[2 * k + b], 16)
        if FINAL_WAITS:
            for i in range(2 * OUT_CH + 1):
                nc.sync.wait_ge(osem[i], 16)

        # --- Scalar engine: x_b0's second half, then the whole weight
        # stream, issued immediately in consumption order.
        nc.scalar.dma_start(xt0[:, XSPLIT:], xt0b_d[:]).then_inc(xsb, 16)
        for o in order:
            nc.scalar.dma_start(wts[o][:], w_d[o][:]).then_inc(wsem[o], 16)

        # --- Tensor engine: warm-up ladder, then the Toeplitz chains.
        for _ in range(NWARM):
            nc.tensor.matmul(
                pss[0][0][:], warm[:, :128], warm[:, 128:], start=True, stop=True
            )
        for _ in range(NWARM_FINE):
            nc.tensor.matmul(
                pss[0][0][:, :128], warm[:, :128], warm[:, 128:256],
                start=True, stop=True,
            )
        nc.tensor.wait_ge(xsa, 16)
        nc.tensor.wait_ge(xsb, 16)

        def chain(k, b, bank, coloff, colw, xtile_off=0):
            o = order[k]
            cs = blocks[o]
            for i, c in enumerate(cs):
                mm = nc.tensor.matmul(
                    bank[:, coloff : coloff + colw],
                    wslice(o, i),
                    xslice(b, c)[:, xtile_off : xtile_off + colw],
                    start=(i == 0),
                    stop=(i == len(cs) - 1),
                )
                if i == len(cs) - 1:
                    mm.then_inc(mm_done, 1)

        for k, b in sched:
            if b == 0:
                # channel-entry guards, once per channel (b0 runs first)
                nc.tensor.wait_ge(wsem[order[k]], 16)
                if k >= NSLOT:
                    # bank reuse: both copies of channel k-NSLOT drained
                    nc.tensor.wait_ge(
                        copy_done_v,
                        max(pos[(k - NSLOT, 0)], pos[(k - NSLOT, 1)]),
                    )
            if (k, b) == (0, 1):
                nc.tensor.wait_ge(xs1, 16)
            if (k, b) == (KL, 1):
                # two column-piece chains into two banks so piece A's
                # copy+DMA overlaps piece B's chain; B is small (H2) to
                # shrink the post-stream copy+DMA tail
                chain(KL, 1, pss[KL % NSLOT][1], 0, H1, 0)
                # pss[0][1] reuse: ch k-NSLOT+1's b=1 copy drained
                nc.tensor.wait_ge(copy_done_v, pos[(KL - NSLOT + 1, 1)])
                chain(KL, 1, pss[0][1], 0, H2, H1)
                continue
            chain(k, b, pss[k % NSLOT][b], 0, A)

        # --- Vector engine: PSUM -> SBUF bf16 copies.
        for k, b in sched:
            if (k, b) == (KL, 1):
                # pieces into ots[(2KL+1)%4]; slot free once (KL-2,b1)
                # DMA (osem[2(KL-2)+1]) completed
                nc.vector.wait_ge(osem[2 * (KL - 2) + 1], 16)
                for h, (bank, off, w_) in enumerate(
                    ((pss[KL % NSLOT][1], 0, H1), (pss[0][1], H1, H2))
                ):
                    nc.vector.wait_ge(mm_done, pos[(KL, 1)] + h)
                    nc.vector.tensor_copy(
                        ots[(2 * KL + 1) % 4][:, off : off + w_], bank[:, :w_]
                    ).then_inc(copy_done_v, 1)
                continue
            nc.vector.wait_ge(mm_done, pos[(k, b)])
            if k >= 2:
                # out-slot reuse: DMA of copy (k-2, b) complete
                nc.vector.wait_ge(osem[2 * (k - 2) + b], 16)
            nc.vector.tensor_copy(
                ots[(2 * k + b) % 4][:], pss[k % NSLOT][b][:]
            ).then_inc(copy_done_v, 1)

    nc.compile()
    return nc


def _np_dtype(name):
    if name == "bfloat16":
        import ml_dtypes

        return ml_dtypes.bfloat16
    return np.float32


def kernel(inputs, log_omegas):
    global LAST_RESULTS
    from concourse.bass_utils import run_bass_kernel_spmd

    mm_np = _np_dtype(MM_DTYPE)
    x = np.asarray(inputs, dtype=np.float32).reshape(B, T)
    wbank = _build_wbank(log_omegas)
    wbank_s = np.pad(wbank, ((0, 0), (GSHIFT, 0)))  # w'[k'] = w[k'-GSHIFT]
    blocks = _plan_blocks(wbank_s)
    cmax = max(c for cs in blocks for c in cs)
    xcols = A + cmax
    assert xcols * 128 >= PAD + GSHIFT + T, "input padding does not fit block reach"
    wmats = _build_weight_mats(wbank_s, blocks, mm_np)

    # X[b][u, m] = xpad[b][128*m + u], xpad = [PAD+GSHIFT zeros | x | tail zeros]
    xpad = np.zeros((B, xcols * 128), np.float32)
    xpad[:, PAD + GSHIFT : PAD + GSHIFT + T] = x
    xt_all = xpad.reshape(B, xcols, 128).transpose(0, 2, 1)  # [B, 128, xcols]
    xt_core = np.ascontiguousarray(
        xt_all.reshape(NCORES, BPC, 128, xcols)
    ).astype(mm_np)  # [NCORES, BPC, 128, xcols]

    key = (
        tuple(tuple(cs) for cs in blocks),
        xcols, MM_DTYPE, OUT_DTYPE, NWARM, NWARM_FINE, FINAL_WAITS,
    )
    if key not in _NC_CACHE:
        _NC_CACHE[key] = _build_nc_raw(blocks, xcols, MM_DTYPE, OUT_DTYPE)
    nc = _NC_CACHE[key]

    XSPLIT = 272
    in_maps = []
    for i in range(NCORES):
        m = {
            "xt0a": np.ascontiguousarray(xt_core[i][0][:, :XSPLIT]),
            "xt0b": np.ascontiguousarray(xt_core[i][0][:, XSPLIT:]),
            "xt1": np.ascontiguousarray(xt_core[i][1]),
        }
        for o in range(OUT_CH):
            m[f"wt{o}"] = wmats[o]
        in_maps.append(m)

    res = run_bass_kernel_spmd(
        nc, in_maps, list(range(NCORES)), trace=TRACE, **TRACE_KWARGS
    )
    LAST_RESULTS = res

    # y_dev[b_loc, o, p, a] = y[b, o, 128*a + p]
    y = np.empty((B, OUT_CH, T), np.float32)
    for i in range(NCORES):
        arr = np.asarray(res.results[i]["y"], dtype=np.float32)
        for b in range(BPC):
            y[i * BPC + b] = arr[b].transpose(0, 2, 1).reshape(OUT_CH, T)
    return y.reshape(B, OUT_CH, T)


# revision 20
# speedup vs baseline: 1.0407x; 1.0407x over previous
"""Trainium2 Bass kernel for nn_DuhamelLayer (8-channel long-FIR conv1d).

Math: out[b,o,t] = sum_k irf[o,k] * x[b, t+k-pad]  (cross-correlation,
'SAME' padding, pad = MAXK//2).  The conv is recast as a chain of
PSUM-accumulating 128x128 Toeplitz-block matmuls on the TensorEngine:

  t = 128*a + p,  k' = 128*c + (u - p)          (k' = k + GSHIFT)
  out[p, a] = sum_c sum_u M_c[u, p] * X[u, a + c]
  M_c[u, p] = w'[128*c + u - p]                 (dense Toeplitz block)
  X[u, m]   = xpad[128*m + u]                   (partition-fast layout)

GSHIFT=76 aligns the per-channel nonzero tap spans to 128-boundaries,
giving the provably minimal 62 blocks (= sum_o ceil((W_o+127)/128);
each tap k' is touched by both 128-blocks (k'+p)//128 as p sweeps, so
the block cover is forced once the phase is chosen).
Operands are bf16 (PE streams 1 col/cycle @2.4GHz warm, FWL weight
loads overlap the previous matmul); PSUM accumulates fp32; the output
is stored bf16 and widened on the host (rel_l2 vs fp64 ~3e-3).
Sharding: data-parallel over batch, 2 batches per core x 8 cores.

Schedule notes (from NTFF traces):
- No nc.Block: all five engine streams are straight-line code in the
  root basic block.  This removes the Block-exit all-engine barrier,
  letting each engine fall into the walrus epilogue (which clears a
  ~50-semaphore stripe per engine, ~2-5.5us!) as soon as its own
  stream ends, overlapping the other engines' tails.  All kernel
  semaphores that receive DMA receipts land in the DVE/Sync stripes,
  whose sweeps start only after those engines' streams end, so no
  receipt can race its own stripe's clear into a deadlock.
- Input x rides the Sync HWDGE ring; ALL weights ride the Scalar
  HWDGE ring, issued immediately (the two rings share the 16 SDMA
  engines at packet granularity, so x_b0 and the first weights stream
  concurrently).  Real matmuls start ~1.7us earlier than with the
  bundled single-ring scheme, and the mid-stream weight-arrival
  stalls at the ch5/ch4 boundaries disappear.
- NWARM warm-up matmuls on uninitialized SBUF bridge the input-DMA
  wait and un-gate the PE-HAM clock (cold 1.2GHz -> warm 2.4GHz after
  ~3.4us of busy PE).
"""

import numpy as np

# ---- static config (mirrors the nn.Module) ----
OMEGAS = [5.0, 7.0, 9.0, 12.0, 16.0, 22.0, 30.0, 40.0]
XI = 0.05
DT = 0.01
UJ_U1 = 0.01

_decay = (1.0 / (2.0 * np.pi * XI)) * np.log(1.0 / UJ_U1)
VALID_W = [int(2.0 * np.pi / w / np.sqrt(1.0 - XI**2) * _decay / DT) for w in OMEGAS]
KER = [2 * a - 1 for a in VALID_W]
MAXK = max(KER)          # 3687
OUT_CH = len(OMEGAS)     # 8
PAD = MAXK // 2          # 1843

B = 16                   # batch
T = 65536                # sequence length
NCORES = 8
BPC = B // NCORES        # 2 batches per core
A = T // 128             # 512 output columns per (b, o) tile

GSHIFT = 76              # global tap shift: minimizes total Toeplitz blocks
MM_DTYPE = "bfloat16"    # "bfloat16" | "float32r" | "float32"
OUT_DTYPE = "bfloat16"   # device-side output dtype ("bfloat16" | "float32")
MODE = "raw"             # kept for test.py compat
NWARM = 3                # coarse warm-up matmuls (N=512); see NWARM_FINE
NWARM_FINE = 14          # fine warm-up matmuls (N=128) after the coarse ones:
                         # ~107ns granularity so the PE never idles between
                         # warm-up end and input arrival — ANY sub-us PE gap
                         # before the HAM flip postpones the 2.4GHz clock by
                         # up to a whole 3.4us window (costs ~1.5-3us)
FINAL_WAITS = False      # wait out-DMA receipts before stream end (walrus
                         # postamble dma-drain covers them)
TRACE = False            # test.py flips this for profiling
TRACE_KWARGS = {}
LAST_RESULTS = None

_NC_CACHE = {}


def _build_wbank(log_omegas):
    """float32 numpy mirror of the reference's _build_irfs -> [OUT_CH, MAXK]."""
    lo = np.asarray(log_omegas, dtype=np.float32)
    omegas = np.clip(np.exp(lo), 0.01, 1000.0).astype(np.float32)
    sq = np.float32(np.sqrt(np.float32(1.0 - XI**2)))
    rows = []
    for i in range(OUT_CH):
        W, K = VALID_W[i], KER[i]
        tt = (np.arange(W, dtype=np.float32) * np.float32(DT)).astype(np.float32)
        omegaD = np.float32(omegas[i] * sq)
        irf = (
            (np.float32(1.0) / omegaD)
            * np.exp((-np.float32(XI) * omegas[i]) * tt)
            * np.sin(omegaD * tt)
        ).astype(np.float32)
        w = np.concatenate([irf[::-1], np.zeros((K // 2,), np.float32)])
        addpad = MAXK - K
        w = np.pad(w, (addpad // 2, addpad // 2))
        rows.append(w)
    return np.stack(rows)


def _plan_blocks(wbank_s):
    """Per channel, the Toeplitz block indices c spanning the nonzero taps."""
    blocks = []
    for o in range(OUT_CH):
        nz = np.nonzero(wbank_s[o])[0]
        kmin, kmax = int(nz.min()), int(nz.max())
        blocks.append(list(range(kmin // 128, (kmax + 127) // 128 + 1)))
    return blocks


def _build_weight_mats(wbank_s, blocks, np_dtype):
    """Per channel: [128, nblk*128] with column block i = M_{c_i}[u, p]."""
    maxk = wbank_s.shape[1]
    u = np.arange(128)[:, None]
    p = np.arange(128)[None, :]
    mats = []
    for o in range(OUT_CH):
        cols = []
        for c in blocks[o]:
            idx = 128 * c + u - p
            valid = (idx >= 0) & (idx < maxk)
            cols.append(
                np.where(valid, wbank_s[o][np.clip(idx, 0, maxk - 1)], np.float32(0.0))
            )
        mats.append(
            np.ascontiguousarray(np.concatenate(cols, axis=1)).astype(np_dtype)
        )
    return mats


def _build_nc_raw(blocks, xcols, mm_dtype, out_dtype):
    """Straight-line raw bacc kernel: DMA in, Toeplitz matmul chain, DMA out.

    No nc.Block: per-engine streams are emitted directly into the root
    basic block so no exit barrier separates the body from the walrus
    epilogue (see module docstring).
    """
    import concourse.bacc as bacc
    import concourse.mybir as mybir

    mm_dt = getattr(mybir.dt, mm_dtype)
    out_dt = getattr(mybir.dt, out_dtype)
    f32 = mybir.dt.float32

    nc = bacc.Bacc("TRN2", target_bir_lowering=False, debug=False)
    order = sorted(range(OUT_CH), key=lambda o: len(blocks[o]))

    # x batch 0 is the critical-path input: split it across BOTH HWDGE
    # rings (sync + scalar) so the two halves transfer concurrently.
    XSPLIT = 272
    xt0a_d = nc.dram_tensor("xt0a", [128, XSPLIT], mm_dt, kind="ExternalInput")
    xt0b_d = nc.dram_tensor("xt0b", [128, xcols - XSPLIT], mm_dt, kind="ExternalInput")
    xt1_d = nc.dram_tensor("xt1", [128, xcols], mm_dt, kind="ExternalInput")
    o_first = order[0]
    nbf = len(blocks[o_first])
    # first channel's weights split at block granularity: the first matmul
    # only needs block 0, so the chain can start before blocks 1.. land.
    w0a_d = nc.dram_tensor("wt0a", [128, 128], mm_dt, kind="ExternalInput")
    w0b_d = nc.dram_tensor("wt0b", [128, (nbf - 1) * 128], mm_dt, kind="ExternalInput")
    w_d = {
        o: nc.dram_tensor(f"wt{o}", [128, len(blocks[o]) * 128], mm_dt, kind="ExternalInput")
        for o in range(OUT_CH)
        if o != o_first
    }
    y_d = nc.dram_tensor("y", [BPC, OUT_CH, 128, A], out_dt, kind="ExternalOutput")

    NSLOT = 4  # psum slots; slot s holds banks (b0, b1) of channel k=s mod 4

    from contextlib import ExitStack

    with ExitStack() as ctx:
        xt0 = ctx.enter_context(nc.sbuf_tensor("xt0s", [128, xcols], mm_dt))
        xt1 = ctx.enter_context(nc.sbuf_tensor("xt1s", [128, xcols], mm_dt))
        warm = ctx.enter_context(nc.sbuf_tensor("warms", [128, 128 + A], mm_dt))
        wts = {
            o: ctx.enter_context(
                nc.sbuf_tensor(f"wts{o}", [128, len(blocks[o]) * 128], mm_dt)
            )
            for o in range(OUT_CH)
        }
        ots = [
            ctx.enter_context(nc.sbuf_tensor(f"ots{j}", [128, A], out_dt))
            for j in range(4)
        ]
        pss = [
            [
                ctx.enter_context(nc.psum_tensor(f"rps{s}_{b}", [128, A], f32))
                for b in range(BPC)
            ]
            for s in range(NSLOT)
        ]

        def wslice(o, i):
            return wts[o][:, i * 128 : (i + 1) * 128]

        def xslice(b, c):
            xt = xt0 if b == 0 else xt1
            return xt[:, c : c + A]

        # one semaphore per DMA: the 16 SDMA engines complete their shares of
        # successive same-ring DMAs out of order, so cumulative thresholds on
        # a shared semaphore do NOT imply per-DMA completion.
        xsa = ctx.enter_context(nc.semaphore("xsa"))
        xsb = ctx.enter_context(nc.semaphore("xsb"))
        xs1 = ctx.enter_context(nc.semaphore("xs1"))
        ws0a = ctx.enter_context(nc.semaphore("ws0a"))
        ws0b = ctx.enter_context(nc.semaphore("ws0b"))
        wsem = {
            o: ctx.enter_context(nc.semaphore(f"ws{o}"))
            for o in range(OUT_CH)
            if o != o_first
        }
        osem = [
            ctx.enter_context(nc.semaphore(f"os{i}")) for i in range(2 * OUT_CH + 1)
        ]
        mm_done = ctx.enter_context(nc.semaphore("mm_done"))
        copy_done_v = ctx.enter_context(nc.semaphore("copy_done_v"))

        KL = OUT_CH - 1
        H1 = 384   # first piece of the final half-chain (overlaps last copy)
        H2 = A - H1  # small final piece -> short last copy + out-DMA tail

        # chain schedule: the first two channels' b0 chains run before any
        # b1 chain, buying the xs1 (x_b1) receipt extra slack.  mm_done /
        # copy_done_v counts follow this order on every engine.
        sched = [(0, 0), (1, 0), (0, 1), (1, 1)]
        for _k in range(2, OUT_CH):
            sched += [(_k, 0), (_k, 1)]
        pos = {kb: i + 1 for i, kb in enumerate(sched)}

        # --- Sync engine: x input DMAs, then output DMAs gated on copies.
        # xt1 is held until xt0a's receipt so its transfer doesn't steal
        # shared-SDMA bandwidth from the critical set (x_b0 + wt0 block 0).
        nc.sync.dma_start(xt0[:, :XSPLIT], xt0a_d[:]).then_inc(xsa, 16)
        nc.sync.wait_ge(xsa, 16)
        nc.sync.dma_start(xt1[:], xt1_d[:]).then_inc(xs1, 16)
        for k, b in sched:
            if (k, b) == (KL, 1):
                # last channel's b=1 pieces (copies are incs 16 and 17)
                for h, (off, w_, os_i) in enumerate(
                    ((0, H1, 2 * KL + 1), (H1, H2, 2 * OUT_CH))
                ):
                    nc.sync.wait_ge(copy_done_v, pos[(KL, 1)] + h)
                    nc.sync.dma_start(
                        y_d[1, order[KL]][:, off : off + w_],
                        ots[(2 * KL + 1) % 4][:, off : off + w_],
                    ).then_inc(osem[os_i], 16)
                continue
            nc.sync.wait_ge(copy_done_v, pos[(k, b)])
            nc.sync.dma_start(
                y_d[b, order[k]], ots[(2 * k + b) % 4][:]
            ).then_inc(osem[2 * k + b], 16)
        if FINAL_WAITS:
            for i in range(2 * OUT_CH + 1):
                nc.sync.wait_ge(osem[i], 16)

        # --- Scalar engine: x_b0's second half, then the weight stream in
        # consumption order, first channel at block granularity.
        nc.scalar.dma_start(xt0[:, XSPLIT:], xt0b_d[:]).then_inc(xsb, 16)
        nc.scalar.dma_start(wts[o_first][:, :128], w0a_d[:]).then_inc(ws0a, 16)
        nc.scalar.dma_start(wts[o_first][:, 128:], w0b_d[:]).then_inc(ws0b, 16)
        for o in order[1:]:
            nc.scalar.dma_start(wts[o][:], w_d[o][:]).then_inc(wsem[o], 16)

        # --- Tensor engine: warm-up ladder, then the Toeplitz chains.
        for _ in range(NWARM):
            nc.tensor.matmul(
                pss[0][0][:], warm[:, :128], warm[:, 128:], start=True, stop=True
            )
        for _ in range(NWARM_FINE):
            nc.tensor.matmul(
                pss[0][0][:, :128], warm[:, :128], warm[:, 128:256],
                start=True, stop=True,
            )
        nc.tensor.wait_ge(xsa, 16)
        nc.tensor.wait_ge(xsb, 16)

        def chain(k, b, bank, coloff, colw, xtile_off=0, block_waits=None):
            o = order[k]
            cs = blocks[o]
            for i, c in enumerate(cs):
                if block_waits and i in block_waits:
                    nc.tensor.wait_ge(*block_waits[i])
                mm = nc.tensor.matmul(
                    bank[:, coloff : coloff + colw],
                    wslice(o, i),
                    xslice(b, c)[:, xtile_off : xtile_off + colw],
                    start=(i == 0),
                    stop=(i == len(cs) - 1),
                )
                if i == len(cs) - 1:
                    mm.then_inc(mm_done, 1)

        for k, b in sched:
            if b == 0:
                # channel-entry guards, once per channel (b0 runs first)
                if k == 0:
                    nc.tensor.wait_ge(ws0a, 16)
                else:
                    nc.tensor.wait_ge(wsem[order[k]], 16)
                if k >= NSLOT:
                    # bank reuse: both copies of channel k-NSLOT drained
                    nc.tensor.wait_ge(
                        copy_done_v,
                        max(pos[(k - NSLOT, 0)], pos[(k - NSLOT, 1)]),
                    )
            if (k, b) == (0, 1):
                nc.tensor.wait_ge(xs1, 16)
            if (k, b) == (KL, 1):
                # two column-piece chains into two banks so piece A's
                # copy+DMA overlaps piece B's chain; B is small (H2) to
                # shrink the post-stream copy+DMA tail
                chain(KL, 1, pss[KL % NSLOT][1], 0, H1, 0)
                # pss[0][1] reuse: ch k-NSLOT+1's b=1 copy drained
                nc.tensor.wait_ge(copy_done_v, pos[(KL - NSLOT + 1, 1)])
                chain(KL, 1, pss[0][1], 0, H2, H1)
                continue
            bw = {1: (ws0b, 16)} if (k, b) == (0, 0) else None
            chain(k, b, pss[k % NSLOT][b], 0, A, block_waits=bw)

        # --- Vector engine: PSUM -> SBUF bf16 copies.
        for k, b in sched:
            if (k, b) == (KL, 1):
                # pieces into ots[(2KL+1)%4]; slot free once (KL-2,b1)
                # DMA (osem[2(KL-2)+1]) completed
                nc.vector.wait_ge(osem[2 * (KL - 2) + 1], 16)
                for h, (bank, off, w_) in enumerate(
                    ((pss[KL % NSLOT][1], 0, H1), (pss[0][1], H1, H2))
                ):
                    nc.vector.wait_ge(mm_done, pos[(KL, 1)] + h)
                    nc.vector.tensor_copy(
                        ots[(2 * KL + 1) % 4][:, off : off + w_], bank[:, :w_]
                    ).then_inc(copy_done_v, 1)
                continue
            nc.vector.wait_ge(mm_done, pos[(k, b)])
            if k >= 2:
                # out-slot reuse: DMA of copy (k-2, b) complete
                nc.vector.wait_ge(osem[2 * (k - 2) + b], 16)
            nc.vector.tensor_copy(
                ots[(2 * k + b) % 4][:], pss[k % NSLOT][b][:]
            ).then_inc(copy_done_v, 1)

    nc.compile()
    return nc


def _np_dtype(name):
    if name == "bfloat16":
        import ml_dtypes

        return ml_dtypes.bfloat16
    return np.float32


def kernel(inputs, log_omegas):
    global LAST_RESULTS
    from concourse.bass_utils import run_bass_kernel_spmd

    mm_np = _np_dtype(MM_DTYPE)
    x = np.asarray(inputs, dtype=np.float32).reshape(B, T)
    wbank = _build_wbank(log_omegas)
    wbank_s = np.pad(wbank, ((0, 0), (GSHIFT, 0)))  # w'[k'] = w[k'-GSHIFT]
    blocks = _plan_blocks(wbank_s)
    cmax = max(c for cs in blocks for c in cs)
    xcols = A + cmax
    assert xcols * 128 >= PAD + GSHIFT + T, "input padding does not fit block reach"
    wmats = _build_weight_mats(wbank_s, blocks, mm_np)

    # X[b][u, m] = xpad[b][128*m + u], xpad = [PAD+GSHIFT zeros | x | tail zeros]
    xpad = np.zeros((B, xcols * 128), np.float32)
    xpad[:, PAD + GSHIFT : PAD + GSHIFT + T] = x
    xt_all = xpad.reshape(B, xcols, 128).transpose(0, 2, 1)  # [B, 128, xcols]
    xt_core = np.ascontiguousarray(
        xt_all.reshape(NCORES, BPC, 128, xcols)
    ).astype(mm_np)  # [NCORES, BPC, 128, xcols]

    key = (
        tuple(tuple(cs) for cs in blocks),
        xcols, MM_DTYPE, OUT_DTYPE, NWARM, NWARM_FINE, FINAL_WAITS,
    )
    if key not in _NC_CACHE:
        _NC_CACHE[key] = _build_nc_raw(blocks, xcols, MM_DTYPE, OUT_DTYPE)
    nc = _NC_CACHE[key]

    XSPLIT = 272
    order = sorted(range(OUT_CH), key=lambda o: len(blocks[o]))
    o_first = order[0]
    in_maps = []
    for i in range(NCORES):
        m = {
            "xt0a": np.ascontiguousarray(xt_core[i][0][:, :XSPLIT]),
            "xt0b": np.ascontiguousarray(xt_core[i][0][:, XSPLIT:]),
            "xt1": np.ascontiguousarray(xt_core[i][1]),
            "wt0a": np.ascontiguousarray(wmats[o_first][:, :128]),
            "wt0b": np.ascontiguousarray(wmats[o_first][:, 128:]),
        }
        for o in range(OUT_CH):
            if o != o_first:
                m[f"wt{o}"] = wmats[o]
        in_maps.append(m)

    res = run_bass_kernel_spmd(
        nc, in_maps, list(range(NCORES)), trace=TRACE, **TRACE_KWARGS
    )
    LAST_RESULTS = res

    # y_dev[b_loc, o, p, a] = y[b, o, 128*a + p]
    y = np.empty((B, OUT_CH, T), np.float32)
    for i in range(NCORES):
        arr = np.asarray(res.results[i]["y"], dtype=np.float32)
        for b in range(BPC):
            y[i * BPC + b] = arr[b].transpose(0, 2, 1).reshape(OUT_CH, T)
    return y.reshape(B, OUT_CH, T)
